# revision 1
# baseline (speedup 1.0000x reference)
"""Trainium2 Bass kernel for an autoregressive free-run rollout of a small MLP.

Model (per reference.py):
    B=64, C_IN=4, C_OUT=4, T=1024, H=512, RF=32, io_delay=1
    step t:  h = relu(Wu @ u_win[t] + Wy @ y_win[t] + b1);  y[t] = W2 @ h + b2
    u_win[t] = delayed-input window (recurrence-independent)
    y_win[t] = previous RF outputs (sequential dependency)

Sharding: data-parallel over batch across 8 cores (B_LOC=8/core), weights
replicated, zero inter-core communication.

Device algorithm (per core, fully unrolled over T):
  - The u-contribution ("u_proj") for every step is computed by batched
    matmuls directly into PSUM regions ahead of time (6 PSUM banks hold 96
    steps of preactivations; banks are refilled in the shadow of the serial
    chain).
  - Serial chain per step t (s = t mod 32, q = s//8, g = s%8):
      MM1   (PE):  4 chunk matmuls accumulate Wy_rot[s]^T @ y_cbuf onto the
                   preloaded u_proj PSUM region -> full preactivation (128,32)
      relu  (ACT): PSUM -> SBUF h (128, 32), layout [H-chunk-col (c,b)]
      MM2p  (PE):  4 chunk matmuls with W2 padded to 32 columns so y lands at
                   quadrant lanes 4g..4g+3, + 1 "merge" matmul S_g^T @
                   y_cbuf[q] that re-emits the quadrant's other 7 slots ->
                   PSUM quadrant = updated window quadrant
      copy  (ACT): aligned 32-partition copy PSUM -> y_cbuf quadrant
      copy  (DVE): aligned 32-partition copy PSUM -> y_hist (off critical path)
  - y window lives in SBUF as (128, 8): partition 4s+o = slot s, channel o.
    Slot->delay mapping rotates with t; 32 pre-rotated copies of Wy handle it.
  - Output staging y_hist (128, 8*T): quadrant-sized stripes; host extracts
    the 4 meaningful lanes per step and reassembles (B, C_OUT, T).
"""

import numpy as np

import concourse.bacc as bacc
import concourse.mybir as mybir
from concourse import bass_utils
from concourse.tile import TileContext

# Problem constants (hardcoded per contract).
B_FULL, C_IN, C_OUT, T, H, RF = 64, 4, 4, 1024, 512, 32
IO_DELAY = 1
N_CORES = 8
B_LOC = B_FULL // N_CORES          # 8
NCH = H // 128                     # 4 H-chunks
NREG = 16                          # psum regions (steps) per bank
NBANK = 6                          # psum banks holding u_proj preactivations
NYQ = 2                            # rotating psum tiles for the y quadrant
F32 = mybir.dt.float32

_cache = {}
FB_BF16 = True  # bf16 feedback path (Wy, y window, W2, h); fp32 u-path


def _build(T_steps, b2_any=False, fb_bf16=False, reps=1):
    """Build the Bacc program (SPMD, identical on all cores)."""
    nc = bacc.Bacc("TRN2", target_bir_lowering=False, debug=False,
                   num_devices=N_CORES)

    DT = mybir.dt.bfloat16 if fb_bf16 else F32
    d_u = nc.dram_tensor("u_lay", [128, T_steps * B_LOC], F32,
                         kind="ExternalInput").ap()
    d_wyrot = nc.dram_tensor("wy_rot", [128, 32 * NCH * 128], DT,
                             kind="ExternalInput").ap()
    d_wut = nc.dram_tensor("wu_t", [128, H], F32, kind="ExternalInput").ap()
    d_w2p = nc.dram_tensor("w2_qpad", [128, 8 * NCH * 32], DT,
                           kind="ExternalInput").ap()
    d_sg = nc.dram_tensor("s_merge", [128, 8 * 32], DT,
                          kind="ExternalInput").ap()
    d_b2m = nc.dram_tensor("b2_mask", [128, 8], F32,
                           kind="ExternalInput").ap()
    d_out = nc.dram_tensor("y_hist", [128, B_LOC * T_steps], F32,
                           kind="ExternalOutput").ap()

    n_windows = T_steps // NREG
    assert T_steps % NREG == 0

    with TileContext(nc) as tc:
        with (
            tc.tile_pool(name="const", bufs=1) as cpool,
            tc.tile_pool(name="hp", bufs=8) as hpool,
            tc.tile_pool(name="ph", bufs=1, space="PSUM") as ppool,
            tc.tile_pool(name="py", bufs=1, space="PSUM") as pypool,
        ):
            U = cpool.tile([128, T_steps * B_LOC], F32, tag="U")
            WyR = cpool.tile([128, 32 * NCH * 128], DT, tag="WyR")
            WuT = cpool.tile([128, H], F32, tag="WuT")
            W2P = cpool.tile([128, 8 * NCH * 32], DT, tag="W2P")
            SG = cpool.tile([128, 8 * 32], DT, tag="SG")
            B2M = cpool.tile([128, 8], F32, tag="B2M")
            ycb = cpool.tile([128, B_LOC], DT, tag="ycb")
            yhist = cpool.tile([128, B_LOC * T_steps], F32, tag="yhist")

            # Split big input DMAs: the chain's first steps depend only on
            # the first slices, so compute starts ~30us earlier.
            upiece = T_steps * B_LOC // 4
            for i in range(4):
                nc.sync.dma_start(U[:, i * upiece:(i + 1) * upiece],
                                  d_u[:, i * upiece:(i + 1) * upiece])
            for i in range(4):
                nc.sync.dma_start(WyR[:, i * 4096:(i + 1) * 4096],
                                  d_wyrot[:, i * 4096:(i + 1) * 4096])
            nc.sync.dma_start(WuT[:], d_wut)
            nc.sync.dma_start(W2P[:], d_w2p)
            nc.sync.dma_start(SG[:], d_sg)
            nc.sync.dma_start(B2M[:], d_b2m)
            pbank = [ppool.tile([128, NREG * 32], F32, tag=f"pb{w}",
                                name=f"pb{w}") for w in range(NBANK)]
            pyq = [pypool.tile([128, B_LOC], F32, tag=f"py{i}",
                               name=f"py{i}") for i in range(NYQ)]

            def preload(window, chunk):
                """u_proj matmul (one H-chunk) for the 16 steps of `window`
                into bank window%NBANK.

                start=True clears has_written for the WHOLE bank, so only
                chunk 0 starts; chunks 1-3 write with cleared bits (fresh
                overwrite) and set them, letting the later MM1 accumulate."""
                bank = pbank[window % NBANK]
                rhs = U[:, window * NREG * B_LOC:(window + 1) * NREG * B_LOC]
                out = bank[:].rearrange("p (r cb) -> p r cb", cb=32)[
                    :, :, chunk * B_LOC:(chunk + 1) * B_LOC]
                nc.tensor.matmul(out, WuT[:, chunk * 128:(chunk + 1) * 128],
                                 rhs, start=(chunk == 0), stop=False,
                                 skip_group_check=True)

            for rep in range(reps):
              nc.gpsimd.memset(ycb[:], 0.0)
              for w in range(min(NBANK, n_windows)):
                for c in range(NCH):
                    preload(w, c)

              for t in range(T_steps):
                  s = t % 32
                  q = s // 8
                  g = s % 8
                  w = (t // NREG) % NBANK
                  reg = t % NREG
                  region = pbank[w][:, reg * 32:(reg + 1) * 32]

                  # MM1: y-window contribution accumulated onto u_proj.
                  for c in range(NCH):
                      nc.tensor.matmul(
                          region[:, c * B_LOC:(c + 1) * B_LOC],
                          WyR[:, (s * NCH + c) * 128:(s * NCH + c + 1) * 128],
                          ycb[:],
                          start=False, stop=True, skip_group_check=True)

                  # relu -> SBUF h (bias b1 == 0 for this problem).
                  h = hpool.tile([128, NCH * B_LOC], DT, tag="h")
                  nc.scalar.activation(h[:], region,
                                       mybir.ActivationFunctionType.Relu)

                  # MM2 padded + merge matmul -> psum quadrant q.
                  # (Tried merge-first and a split relu: both schedule worse on
                  # HW — this order measured fastest.)
                  ysc = pyq[t % NYQ]
                  oquad = ysc[32 * q:32 * (q + 1), :]
                  for c in range(NCH):
                      nc.tensor.matmul(
                          oquad,
                          W2P[:, (g * NCH + c) * 32:(g * NCH + c + 1) * 32],
                          h[:, c * B_LOC:(c + 1) * B_LOC],
                          start=(c == 0), stop=False,
                          tile_position=(0, 32 * q), skip_group_check=True)
                  nc.tensor.matmul(
                      oquad,
                      SG[32 * q:32 * (q + 1), g * 32:(g + 1) * 32],
                      ycb[32 * q:32 * (q + 1), :],
                      start=False, stop=True,
                      tile_position=(32 * q, 32 * q), skip_group_check=True)

                  # Updated quadrant -> y window (ACT, on chain) and
                  # -> output staging (DVE, off chain). b2 == 0 here; the
                  # masked bias B2M handles nonzero b2 generically.
                  if b2_any:
                      nc.vector.tensor_scalar_add(
                          ycb[32 * q:32 * (q + 1), :], oquad,
                          B2M[32 * q:32 * (q + 1), g:g + 1])
                      nc.scalar.activation(
                          yhist[32 * q:32 * (q + 1), t * B_LOC:(t + 1) * B_LOC],
                          oquad, mybir.ActivationFunctionType.Identity,
                          bias=B2M[32 * q:32 * (q + 1), g:g + 1])
                  else:
                      nc.vector.tensor_copy(
                          ycb[32 * q:32 * (q + 1), :], oquad)
                      nc.scalar.activation(
                          yhist[32 * q:32 * (q + 1), t * B_LOC:(t + 1) * B_LOC],
                          oquad, mybir.ActivationFunctionType.Copy)

                  # Stream completed yhist columns to DRAM in the shadow of
                  # the chain; only the last piece remains after the loop.
                  if (t + 1) % 128 == 0 or t == T_steps - 1:
                      pst = (t // 128) * 128
                      nc.sync.dma_start(
                          d_out[:, pst * B_LOC:(t + 1) * B_LOC],
                          yhist[:, pst * B_LOC:(t + 1) * B_LOC])

                  # Refill: during steps 0..3 of window cw, emit one chunk of
                  # the preload for window cw+5 (its bank was freed when window
                  # cw-1 finished).  Fills the PE gap while the insert copy
                  # round-trips.
                  if reg % 4 == 0:
                      target = t // NREG + NBANK - 1
                      if NBANK <= target < n_windows:
                          preload(target, reg // 4)

    nc.compile()
    return nc


_wcache = {}


def _host_prep(u_core, W1, b1, W2, b2, T_steps, fb_bf16=False):
    """Build per-core input arrays in device layouts (pure layout work).

    Weight layouts are identical across cores (only u differs), so they are
    memoized on a content digest."""
    import hashlib
    wkey = (hashlib.sha1(np.ascontiguousarray(W1).tobytes()
                         + np.ascontiguousarray(W2).tobytes()
                         + np.ascontiguousarray(b2).tobytes()).hexdigest(),
            T_steps, fb_bf16)
    cached = _wcache.get(wkey)
    if cached is not None:
        out = dict(cached)
        out["u_lay"] = _prep_u(u_core, T_steps)
        return out
    H_, CM, RF_ = W1.shape
    Wu = np.ascontiguousarray(W1[:, :C_IN, :]).reshape(H, C_IN * RF)
    Wy = np.ascontiguousarray(W1[:, C_IN:, :])          # (H, C_OUT, RF)

    u_lay = _prep_u(u_core, T_steps)

    # wu_t[ck, j] = Wu[j, ck]
    wu_t = np.ascontiguousarray(Wu.T)                   # (128, 512)

    # wy_rot[(4s+o), (s_rot*NCH + c)*128 + j'] = Wy[128c+j', o, RF-d(s, r)]
    wy_rot = np.zeros((128, 32 * NCH * 128), np.float32)
    s_idx = np.arange(32)
    for r in range(32):
        d = ((r - s_idx - 1) % 32) + 1                  # delay of slot s at r
        k = RF - d                                      # (32,)
        # block (128, 512): rows 4s+o, cols c*128+j'
        blk = Wy[:, :, k]                               # (H, C_OUT, 32) [s]
        blk = blk.transpose(2, 1, 0).reshape(128, H)    # rows (s,o), cols j
        wy_rot[:, r * NCH * 128:(r + 1) * NCH * 128] = blk
    # columns within rotation r are already c-major (j = 128c + j')

    # w2_qpad[p, (g*NCH+c)*32 + m] = W2[o, 128c+p] if m == 4g+o else 0
    w2_qpad = np.zeros((128, 8 * NCH * 32), np.float32)
    for g in range(8):
        for c in range(NCH):
            blk = np.zeros((128, 32), np.float32)
            for o in range(C_OUT):
                blk[:, 4 * g + o] = W2[o, c * 128:(c + 1) * 128]
            w2_qpad[:, (g * NCH + c) * 32:(g * NCH + c + 1) * 32] = blk

    # s_merge[32q+i, g*32+m] = 1 if i == m and not (4g <= i < 4g+4) else 0
    s_merge = np.zeros((128, 8 * 32), np.float32)
    eye = np.eye(32, dtype=np.float32)
    for g in range(8):
        m = eye.copy()
        m[4 * g:4 * g + 4, :] = 0.0
        for q in range(4):
            s_merge[32 * q:32 * (q + 1), g * 32:(g + 1) * 32] = m

    # b2_mask[32q + i, g] = b2[i - 4g] if 4g <= i < 4g+4 else 0
    b2_mask = np.zeros((128, 8), np.float32)
    for g in range(8):
        for o in range(C_OUT):
            for q in range(4):
                b2_mask[32 * q + 4 * g + o, g] = b2[o]

    if fb_bf16:
        import ml_dtypes
        bf = ml_dtypes.bfloat16
        wy_rot = wy_rot.astype(bf)
        w2_qpad = w2_qpad.astype(bf)
        s_merge = s_merge.astype(bf)
    _wcache[wkey] = {"wy_rot": wy_rot, "wu_t": wu_t, "w2_qpad": w2_qpad,
                     "s_merge": s_merge, "b2_mask": b2_mask}
    return {"u_lay": u_lay, **_wcache[wkey]}


def _prep_u(u_core, T_steps):
    """u_lay[c*32+k, t*8+b] = u_padded[b, c, t+k]."""
    u_pad = np.zeros((B_LOC, C_IN, T_steps + RF - 1), np.float32)
    if T_steps > IO_DELAY:
        u_pad[:, :, RF:] = u_core[:, :, :T_steps - IO_DELAY]
    win = np.lib.stride_tricks.sliding_window_view(u_pad, T_steps, axis=2)
    # win[b, c, k, t] = u_pad[b, c, k + t]
    return np.ascontiguousarray(
        win.transpose(1, 2, 3, 0).reshape(128, T_steps * B_LOC))


def _extract(y_hist, T_steps):
    """y_hist (128, 8*T) -> (B_LOC, C_OUT, T)."""
    out = np.empty((B_LOC, C_OUT, T_steps), np.float32)
    t_idx = np.arange(T_steps)
    s = t_idx % 32
    rows = 32 * (s // 8) + 4 * (s % 8)                  # (T,)
    cols = t_idx[:, None] * B_LOC + np.arange(B_LOC)[None, :]   # (T, B)
    for o in range(C_OUT):
        out[:, o, :] = y_hist[(rows + o)[:, None], cols].T
    return out


def kernel(u, W1, b1, W2, b2):
    T_steps = u.shape[2]
    assert not np.asarray(b1).any(), "kernel assumes b1 == 0"
    b2_any = bool(np.asarray(b2).any())
    key = (T_steps, b2_any, FB_BF16)
    if key not in _cache:
        _cache[key] = _build(T_steps, b2_any, FB_BF16)
    nc = _cache[key]

    in_maps = []
    for core in range(N_CORES):
        u_core = np.asarray(u[core * B_LOC:(core + 1) * B_LOC],
                            dtype=np.float32)
        in_maps.append(_host_prep(u_core, np.asarray(W1), np.asarray(b1),
                                  np.asarray(W2), np.asarray(b2), T_steps,
                                  FB_BF16))

    res = bass_utils.run_bass_kernel_spmd(nc, in_maps,
                                          core_ids=list(range(N_CORES)))
    outs = [_extract(res.results[c]["y_hist"], T_steps)
            for c in range(N_CORES)]
    return np.concatenate(outs, axis=0)

